# revision 9
# baseline (speedup 1.0000x reference)
"""Trainium2 Bass kernel for a ResNet BasicBlock (stride-2, downsample) in
BatchNorm training mode.

  out = relu(bn2(conv2(relu(bn1(conv1(x))))) + bnd(convd(x)))
  conv1: 3x3 s2 SAME, conv2: 3x3 s1 SAME, convd: 1x1 s2 VALID
  x: (128, 64, 56, 56) f32 -> out: (128, 128, 28, 28) f32

Sharding: data-parallel over batch across 8 NeuronCores (16 images each),
weights replicated.  BN1 uses per-shard batch stats (sanctioned by the
sharding hint; the downstream BN2 re-normalization absorbs most of the
shard-stat error).  BNd and BN2 stats are made exact (full-batch) with one
tiny AllReduce each of the per-core (mean, E[x^2]) vectors.

Convs run as shift-and-accumulate matmuls in bf16 with f32 PSUM
accumulation.  x is pre-packed on the host into an even/odd row- and
column-split layout (zero padding baked in) so every tap's moving operand
is contiguous in its innermost dim (strided operands stream ~60% slower
and do not register as PE activity for the HAM clock un-throttle) and the
(kh=0, kh=1) tap pairs contract over K=128.  Matmuls are ordered
taps-outer over image pairs so consecutive matmuls share the stationary
operand (weight reloads amortize).
"""

import os
import sys

import numpy as np

try:
    import concourse.bass as bass
except ImportError:  # fall back to the staged repo location
    for _p in ("/opt/trn_rl_repo", "/root/.axon_site/_ro/trn_rl_repo"):
        if _p not in sys.path:
            sys.path.insert(0, _p)
    import concourse.bass as bass

import ml_dtypes
import concourse.bacc as bacc
import concourse.mybir as mybir
import concourse.tile as tile
from concourse import bass_utils

F32 = mybir.dt.float32
BF16 = mybir.dt.bfloat16
BF16NP = ml_dtypes.bfloat16

N_CORES = 8
B, CIN, H, W = 128, 64, 56, 56
COUT, OH, OW = 128, 28, 28
PER = B // N_CORES          # images per core
XFREE = 29 * 58             # row-split block: 29 rows x (2 parities x 29 x)
NPIX = OH * OW              # 784
NBLK = 392                  # one half-image block: 14 rows x 28 cols
NB = 2 * PER                # stat blocks per conv (two per image)
Y1F = 30 * 30               # padded y1 layout
EPS = 1e-5

_ADD = mybir.AluOpType.add
_MULT = mybir.AluOpType.mult
_RELU = mybir.ActivationFunctionType.Relu
_GROUPS = [list(range(N_CORES))]


def _kernel_body(tc, nc, xin, wts, gb, out):
    with tc.tile_pool(name="const", bufs=1) as constp, \
         tc.tile_pool(name="xs", bufs=4) as xpool, \
         tc.tile_pool(name="c1p", bufs=PER) as c1pool, \
         tc.tile_pool(name="cdp", bufs=PER) as cdpool, \
         tc.tile_pool(name="c2p", bufs=PER) as c2pool, \
         tc.tile_pool(name="y1p", bufs=PER) as y1pool, \
         tc.tile_pool(name="zfp", bufs=3) as zpool, \
         tc.tile_pool(name="ogp", bufs=3) as opool, \
         tc.tile_pool(name="dram", bufs=1, space="DRAM") as drp:

        w_t = constp.tile([128, 2048], BF16, tag="w")
        nc.sync.dma_start(w_t[:], wts[:])
        gb_t = constp.tile([128, 8], F32, tag="gb")
        nc.sync.dma_start(gb_t[:], gb[:])

        stats1 = constp.tile([128, 6 * NB], F32, tag="st1")
        statsd = constp.tile([128, 6 * NB], F32, tag="std")
        stats2 = constp.tile([128, 6 * NB], F32, tag="st2")
        coef = constp.tile([128, 24], F32, tag="coef")

        def w01(t):
            return w_t[:, t * 128:(t + 1) * 128]

        def wk2(t):
            return w_t[0:64, (3 + t) * 128:(4 + t) * 128]

        wdk = w_t[0:64, 6 * 128:7 * 128]

        def w2k(kh, kw):
            t = 7 + 3 * kh + kw
            return w_t[:, t * 128:(t + 1) * 128]

        c1_t, cd_t, c2_t, y1_t = [], [], [], []

        # y1 tiles are persistent and zero-padded once; the BN1 activation
        # only ever writes the 28x28 interior, so the pad ring stays zero.
        for n in range(PER):
            y1n = y1pool.tile([128, Y1F], BF16, tag="y1")
            y1_t.append(y1n)
            nc.gpsimd.memset(y1n[:], 0.0)

        # conv1 taps: (weight AP, partition count, rhs slice builder).
        # x4 dims: [p, row(29), parity(2), x(29)] -- row 28 / x 28 are pads.
        def c1_taps():
            return [
                (w01(0), 128, lambda x4, y0: x4[:, y0:y0 + 14, 0, 0:28]),
                (w01(1), 128, lambda x4, y0: x4[:, y0:y0 + 14, 1, 0:28]),
                (w01(2), 128, lambda x4, y0: x4[:, y0:y0 + 14, 0, 1:29]),
                (wk2(0), 64,
                 lambda x4, y0: x4[0:64, y0 + 1:y0 + 15, 0, 0:28]),
                (wk2(1), 64,
                 lambda x4, y0: x4[0:64, y0 + 1:y0 + 15, 1, 0:28]),
                (wk2(2), 64,
                 lambda x4, y0: x4[0:64, y0 + 1:y0 + 15, 0, 1:29]),
            ]

        # ---------------- phase A: conv1, then convd ----------------
        with tc.tile_pool(name="pc1", bufs=6, space="PSUM") as pc1:
            for n0 in range(0, PER, 2):
                pair = (n0, n0 + 1)
                x4s, pss = {}, {}
                for n in pair:
                    xt = xpool.tile([128, XFREE], BF16, tag="xt")
                    nc.sync.dma_start(xt[:], xin[n * 128:(n + 1) * 128, :])
                    x4s[n] = xt.rearrange("p (r t x) -> p r t x",
                                          r=29, t=2, x=29)
                    c1_t.append(c1pool.tile([128, NPIX], BF16, tag="c1", name=f"c1_{n}"))
                    cd_t.append(cdpool.tile([128, NPIX], BF16, tag="cd", name=f"cd_{n}"))

                blocks = [(n, h) for n in pair for h in range(2)]
                for nh in blocks:
                    pss[nh] = pc1.tile([128, NBLK], F32, tag="pc1",
                                       name=f"ps1_{nh[0]}_{nh[1]}")
                # taps outer, blocks inner: consecutive matmuls share lhsT
                taps = c1_taps()
                for t, (w_ap, kp, rhs_fn) in enumerate(taps):
                    for (n, h) in blocks:
                        nc.tensor.matmul(pss[(n, h)], w_ap,
                                         rhs_fn(x4s[n], 14 * h),
                                         start=(t == 0),
                                         stop=(t == len(taps) - 1))
                for (n, h) in blocks:
                    y0 = 14 * h
                    blk = 2 * n + h
                    dst = c1_t[n][:, y0 * 28:(y0 + 14) * 28]
                    nc.scalar.copy(dst, pss[(n, h)][:])
                    nc.vector.bn_stats(stats1[:, 6 * blk:6 * blk + 6], dst)

        # convd: all 32 blocks in one run over a single stationary weight.
        # Sits at the phase A/B boundary so the PE stays busy (HAM stays
        # un-throttled) while DVE/ACT run the BN1 coefficient chain and the
        # first y1 activations.  The even-row/even-col quarter of x is
        # re-fetched from DRAM (the phase A x tiles have been recycled).
        xin4 = xin.rearrange("p (r t x) -> p r t x", r=29, t=2, x=29)
        with tc.tile_pool(name="pcd", bufs=6, space="PSUM") as pcd, \
             tc.tile_pool(name="xdp", bufs=4) as xdpool:
            for n in range(PER):
                xd = xdpool.tile([64, 29 * 29], BF16, tag="xd",
                                 name=f"xd_{n}")
                nc.sync.dma_start(xd[:],
                                  xin4[n * 128:n * 128 + 64, :, 0, :])
                xd3 = xd.rearrange("p (r x) -> p r x", x=29)
                psd = {h: pcd.tile([128, NBLK], F32, tag="pcd",
                                   name=f"psd_{n}_{h}")
                       for h in range(2)}
                for h in range(2):
                    nc.tensor.matmul(psd[h], wdk,
                                     xd3[:, 14 * h:14 * h + 14, 0:28],
                                     start=True, stop=True)
                for h in range(2):
                    y0 = 14 * h
                    blk = 2 * n + h
                    dst = cd_t[n][:, y0 * 28:(y0 + 14) * 28]
                    if h == 0:
                        nc.vector.tensor_copy(dst, psd[h][:])
                    else:
                        nc.scalar.copy(dst, psd[h][:])
                    nc.vector.bn_stats(statsd[:, 6 * blk:6 * blk + 6],
                                       dst)

        # ---- BN1 coefficients (per-shard stats, no sync) ----
        mv1 = coef[:, 0:2]
        nc.vector.bn_aggr(mv1, stats1[:])
        nc.vector.tensor_scalar_add(coef[:, 2:3], mv1[:, 1:2], EPS)
        nc.scalar.sqrt(coef[:, 3:4], coef[:, 2:3])
        nc.vector.reciprocal(coef[:, 4:5], coef[:, 3:4])      # inv1
        s1 = coef[:, 5:6]
        t1 = coef[:, 6:7]
        nc.vector.tensor_mul(s1, gb_t[:, 0:1], coef[:, 4:5])
        nc.vector.tensor_mul(coef[:, 7:8], mv1[:, 0:1], s1)
        nc.vector.tensor_sub(t1, gb_t[:, 1:2], coef[:, 7:8])

        # ---- BNd stats -> AllReduce (overlaps phase B) ----
        mvd = coef[:, 8:10]
        nc.vector.bn_aggr(mvd, statsd[:])
        ar_d = constp.tile([128, 2], F32, tag="ard")
        nc.vector.tensor_copy(ar_d[:, 0:1], mvd[:, 0:1])
        nc.vector.tensor_mul(ar_d[:, 1:2], mvd[:, 0:1], mvd[:, 0:1])
        nc.vector.tensor_add(ar_d[:, 1:2], ar_d[:, 1:2], mvd[:, 1:2])
        bd_in = drp.tile([128, 2], F32, tag="bdi")
        bd_out = drp.tile([128, 2], F32, addr_space="Shared", tag="bdo")
        nc.sync.dma_start(bd_in[:], ar_d[:])
        nc.gpsimd.collective_compute(
            "AllReduce", _ADD, replica_groups=_GROUPS,
            ins=[bd_in.opt()], outs=[bd_out.opt()])
        ard_g = constp.tile([128, 2], F32, tag="ardg")
        nc.sync.dma_start(ard_g[:], bd_out[:])
        nc.vector.tensor_scalar_mul(ard_g[:], ard_g[:], 1.0 / N_CORES)
        nc.vector.tensor_mul(coef[:, 10:11], ard_g[:, 0:1], ard_g[:, 0:1])
        nc.vector.tensor_sub(coef[:, 11:12], ard_g[:, 1:2], coef[:, 10:11])
        nc.vector.tensor_scalar_add(coef[:, 12:13], coef[:, 11:12], EPS)
        nc.scalar.sqrt(coef[:, 13:14], coef[:, 12:13])
        nc.vector.reciprocal(coef[:, 14:15], coef[:, 13:14])  # invd
        sd = coef[:, 15:16]
        td = coef[:, 16:17]
        nc.vector.tensor_mul(sd, gb_t[:, 2:3], coef[:, 14:15])
        nc.vector.tensor_mul(coef[:, 17:18], ard_g[:, 0:1], sd)
        nc.vector.tensor_sub(td, gb_t[:, 3:4], coef[:, 17:18])

        # ---------------- phase B: bn1+relu, conv2 ----------------
        taps9 = [(1, 1)] + [(kh, kw) for kh in range(3)
                            for kw in range(3) if (kh, kw) != (1, 1)]
        with tc.tile_pool(name="pc2", bufs=6, space="PSUM") as pc2:
            for n0 in range(0, PER, 2):
                pair = (n0, n0 + 1)
                yvs = {}
                for n in pair:
                    yv = y1_t[n].rearrange("p (r x) -> p r x", x=30)
                    nc.scalar.activation(yv[:, 1:29, 1:29],
                                         c1_t[n].rearrange(
                                             "p (r x) -> p r x", x=28),
                                         _RELU, bias=t1, scale=s1)
                    yvs[n] = yv
                    c2_t.append(c2pool.tile([128, NPIX], BF16, tag="c2", name=f"c2_{n}"))
                blocks = [(n, h) for n in pair for h in range(2)]
                pss = {nh: pc2.tile([128, NBLK], F32, tag="pc2",
                                    name=f"ps2_{nh[0]}_{nh[1]}")
                       for nh in blocks}
                for t, (kh, kw) in enumerate(taps9):
                    for (n, h) in blocks:
                        y0 = 14 * h
                        rhs = yvs[n][:, y0 + kh:y0 + kh + 14, kw:kw + 28]
                        nc.tensor.matmul(pss[(n, h)], w2k(kh, kw), rhs,
                                         start=(t == 0),
                                         stop=(t == len(taps9) - 1))
                for (n, h) in blocks:
                    y0 = 14 * h
                    blk = 2 * n + h
                    dst = c2_t[n][:, y0 * 28:(y0 + 14) * 28]
                    nc.scalar.copy(dst, pss[(n, h)][:])
                    nc.vector.bn_stats(stats2[:, 6 * blk:6 * blk + 6], dst)

        # v = bnd(cd), in place.  Placed after the conv2 loop so the
        # statically-ordered DVE stream never stalls on the BNd AllReduce
        # mid-phase; by now the collective has long completed.
        for n in range(PER):
            nc.vector.tensor_scalar(cd_t[n][:], cd_t[n][:], sd, td,
                                    _MULT, _ADD)

        # ---- BN2 stats -> AllReduce ----
        mv2 = coef[:, 18:20]
        nc.vector.bn_aggr(mv2, stats2[:])
        ar_2 = constp.tile([128, 2], F32, tag="ar2")
        nc.vector.tensor_copy(ar_2[:, 0:1], mv2[:, 0:1])
        nc.vector.tensor_mul(ar_2[:, 1:2], mv2[:, 0:1], mv2[:, 0:1])
        nc.vector.tensor_add(ar_2[:, 1:2], ar_2[:, 1:2], mv2[:, 1:2])
        b2_in = drp.tile([128, 2], F32, tag="b2i")
        b2_out = drp.tile([128, 2], F32, addr_space="Shared", tag="b2o")
        nc.sync.dma_start(b2_in[:], ar_2[:])
        nc.gpsimd.collective_compute(
            "AllReduce", _ADD, replica_groups=_GROUPS,
            ins=[b2_in.opt()], outs=[b2_out.opt()])
        ar2_g = constp.tile([128, 2], F32, tag="ar2g")
        nc.sync.dma_start(ar2_g[:], b2_out[:])
        nc.vector.tensor_scalar_mul(ar2_g[:], ar2_g[:], 1.0 / N_CORES)
        nc.vector.tensor_mul(coef[:, 20:21], ar2_g[:, 0:1], ar2_g[:, 0:1])
        nc.vector.tensor_sub(coef[:, 21:22], ar2_g[:, 1:2], coef[:, 20:21])
        nc.vector.tensor_scalar_add(coef[:, 22:23], coef[:, 21:22], EPS)
        nc.scalar.sqrt(coef[:, 23:24], coef[:, 22:23])
        s2 = coef[:, 18:19]   # reuse mv2 columns (consumed above)
        t2 = coef[:, 19:20]
        nc.vector.reciprocal(coef[:, 21:22], coef[:, 23:24])  # inv2
        nc.vector.tensor_mul(s2, gb_t[:, 4:5], coef[:, 21:22])
        nc.vector.tensor_mul(coef[:, 20:21], ar2_g[:, 0:1], s2)
        nc.vector.tensor_sub(t2, gb_t[:, 5:6], coef[:, 20:21])

        # ---------------- phase C: combine + relu + store ----------------
        _MAX = mybir.AluOpType.max
        for n in range(PER):
            zf = zpool.tile([128, NPIX], BF16, tag="zf")
            nc.vector.scalar_tensor_tensor(zf[:], c2_t[n][:], s2, cd_t[n][:],
                                           _MULT, _ADD)
            og = opool.tile([128, NPIX], F32, tag="og")
            if n % 2 == 0:
                nc.scalar.activation(og[:], zf[:], _RELU, bias=t2)
            else:
                nc.vector.tensor_scalar(og[:], zf[:], t2, 0.0, _ADD, _MAX)
            nc.sync.dma_start(out[n * 128:(n + 1) * 128, :], og[:])


def build_nc():
    nc = bacc.Bacc("TRN2", target_bir_lowering=False, debug=False,
                   num_devices=N_CORES)
    xin = nc.dram_tensor("xin", [PER * 128, XFREE], BF16,
                         kind="ExternalInput").ap()
    wts = nc.dram_tensor("wts", [128, 2048], BF16, kind="ExternalInput").ap()
    gb = nc.dram_tensor("gb", [128, 8], F32, kind="ExternalInput").ap()
    out = nc.dram_tensor("out", [PER * 128, NPIX], F32,
                         kind="ExternalOutput").ap()
    with tile.TileContext(nc) as tc:
        _kernel_body(tc, nc, xin, wts, gb, out)
    nc.compile()
    return nc


def prep_inputs(x, w1, g1, b1, w2, g2, b2, wd, gd, bd):
    """Host-side shard + layout prep. Returns in_maps for the 8 cores."""
    x = np.asarray(x, dtype=np.float32)
    # even/odd row split on partitions, even/odd column split inside each
    # row: free = [row(29)][parity(2)][x(29)], data rows 0..27 / x 0..27
    xp = np.zeros((B, 128, 29, 2, 29), dtype=np.float32)
    xp[:, 0:64, 0:28, 0, 0:28] = x[:, :, 0::2, 0::2]
    xp[:, 0:64, 0:28, 1, 0:28] = x[:, :, 0::2, 1::2]
    xp[:, 64:128, 0:28, 0, 0:28] = x[:, :, 1::2, 0::2]
    xp[:, 64:128, 0:28, 1, 0:28] = x[:, :, 1::2, 1::2]
    xp = xp.reshape(B, 128, XFREE).astype(BF16NP)

    w1 = np.asarray(w1, dtype=np.float32)
    w2 = np.asarray(w2, dtype=np.float32)
    wd = np.asarray(wd, dtype=np.float32)
    w_all = np.zeros((128, 16, 128), dtype=np.float32)
    for t in range(3):
        w_all[0:64, t, :] = w1[:, :, 0, t].T
        w_all[64:128, t, :] = w1[:, :, 1, t].T
        w_all[0:64, 3 + t, :] = w1[:, :, 2, t].T
    w_all[0:64, 6, :] = wd[:, :, 0, 0].T
    for kh in range(3):
        for kw in range(3):
            w_all[:, 7 + 3 * kh + kw, :] = w2[:, :, kh, kw].T
    w_all = w_all.reshape(128, 2048).astype(BF16NP)

    gbm = np.zeros((128, 8), dtype=np.float32)
    for j, v in enumerate([g1, b1, gd, bd, g2, b2]):
        gbm[:, j] = np.asarray(v, dtype=np.float32)

    in_maps = []
    for c in range(N_CORES):
        shard = xp[c * PER:(c + 1) * PER].reshape(PER * 128, XFREE)
        in_maps.append({"xin": np.ascontiguousarray(shard),
                        "wts": w_all, "gb": gbm})
    return in_maps


_NC_CACHE = None


def kernel(**inputs):
    global _NC_CACHE
    if _NC_CACHE is None:
        _NC_CACHE = build_nc()
    nc = _NC_CACHE
    in_maps = prep_inputs(**inputs)
    res = bass_utils.run_bass_kernel_spmd(
        nc, in_maps, core_ids=list(range(N_CORES)))
    outs = [res.results[c]["out"].reshape(PER, COUT, OH, OW)
            for c in range(N_CORES)]
    return np.ascontiguousarray(np.concatenate(outs, axis=0),
                                dtype=np.float32)


# revision 11
# speedup vs baseline: 1.1724x; 1.1724x over previous
"""Trainium2 Bass kernel for a ResNet BasicBlock (stride-2, downsample) in
BatchNorm training mode.

  out = relu(bn2(conv2(relu(bn1(conv1(x))))) + bnd(convd(x)))
  conv1: 3x3 s2 SAME, conv2: 3x3 s1 SAME, convd: 1x1 s2 VALID
  x: (128, 64, 56, 56) f32 -> out: (128, 128, 28, 28) f32

Sharding: data-parallel over batch across 8 NeuronCores (16 images each),
weights replicated.  BN1 uses per-shard batch stats (sanctioned by the
sharding hint; the downstream BN2 re-normalization absorbs most of the
shard-stat error).  BNd and BN2 stats are made exact (full-batch) with one
tiny AllReduce each of the per-core (mean, E[x^2]) vectors.

Convs run as shift-and-accumulate matmuls in bf16 with f32 PSUM
accumulation.  x is pre-packed on the host into an even/odd row- and
column-split layout (zero padding baked in) so every tap's moving operand
is contiguous in its innermost dim (strided operands stream ~60% slower
and do not register as PE activity for the HAM clock un-throttle) and the
(kh=0, kh=1) tap pairs contract over K=128.  Matmuls are ordered
taps-outer over image pairs so consecutive matmuls share the stationary
operand (weight reloads amortize).
"""

import os
import sys

import numpy as np

try:
    import concourse.bass as bass
except ImportError:  # fall back to the staged repo location
    for _p in ("/opt/trn_rl_repo", "/root/.axon_site/_ro/trn_rl_repo"):
        if _p not in sys.path:
            sys.path.insert(0, _p)
    import concourse.bass as bass

import ml_dtypes
import concourse.bacc as bacc
import concourse.mybir as mybir
import concourse.tile as tile
from concourse import bass_utils

F32 = mybir.dt.float32
BF16 = mybir.dt.bfloat16
BF16NP = ml_dtypes.bfloat16

N_CORES = 8
B, CIN, H, W = 128, 64, 56, 56
COUT, OH, OW = 128, 28, 28
PER = B // N_CORES          # images per core
XFREE = 29 * 58             # row-split block: 29 rows x (2 parities x 29 x)
NPIX = OH * OW              # 784
NBLK = 392                  # one half-image block: 14 rows x 28 cols
NB = 2 * PER                # stat blocks per conv (two per image)
Y1F = 30 * 30               # padded y1 layout
EPS = 1e-5

_ADD = mybir.AluOpType.add
_MULT = mybir.AluOpType.mult
_RELU = mybir.ActivationFunctionType.Relu
_GROUPS = [list(range(N_CORES))]


def _kernel_body(tc, nc, xin, wts, gb, out):
    with tc.tile_pool(name="const", bufs=1) as constp, \
         tc.tile_pool(name="xs", bufs=4) as xpool, \
         tc.tile_pool(name="c1p", bufs=PER) as c1pool, \
         tc.tile_pool(name="cdp", bufs=PER) as cdpool, \
         tc.tile_pool(name="c2p", bufs=PER) as c2pool, \
         tc.tile_pool(name="y1p", bufs=PER) as y1pool, \
         tc.tile_pool(name="zfp", bufs=3) as zpool, \
         tc.tile_pool(name="ogp", bufs=3) as opool, \
         tc.tile_pool(name="dram", bufs=1, space="DRAM") as drp:

        w_t = constp.tile([128, 2048], BF16, tag="w")
        nc.scalar.dma_start(w_t[:], wts[:])
        gb_t = constp.tile([128, 8], F32, tag="gb")
        nc.sync.dma_start(gb_t[:], gb[:])

        stats1 = constp.tile([128, 6 * NB], F32, tag="st1")
        statsd = constp.tile([128, 6 * NB], F32, tag="std")
        stats2 = constp.tile([128, 6 * NB], F32, tag="st2")
        coef = constp.tile([128, 24], F32, tag="coef")

        def w01(t):
            return w_t[:, t * 128:(t + 1) * 128]

        def wk2(t):
            return w_t[0:64, (3 + t) * 128:(4 + t) * 128]

        wdk = w_t[0:64, 6 * 128:7 * 128]

        def w2k(kh, kw):
            t = 7 + 3 * kh + kw
            return w_t[:, t * 128:(t + 1) * 128]

        c1_t, cd_t, c2_t, y1_t = [], [], [], []

        # y1 tiles are persistent and zero-padded once; the BN1 activation
        # only ever writes the 28x28 interior, so the pad ring stays zero.
        for n in range(PER):
            y1n = y1pool.tile([128, Y1F], BF16, tag="y1")
            y1_t.append(y1n)
            nc.gpsimd.memset(y1n[:], 0.0)

        # conv1 taps: (weight AP, partition count, rhs slice builder).
        # x4 dims: [p, row(29), parity(2), x(29)] -- row 28 / x 28 are pads.
        def c1_taps():
            return [
                (w01(0), 128, lambda x4, y0: x4[:, y0:y0 + 14, 0, 0:28]),
                (w01(1), 128, lambda x4, y0: x4[:, y0:y0 + 14, 1, 0:28]),
                (w01(2), 128, lambda x4, y0: x4[:, y0:y0 + 14, 0, 1:29]),
                (wk2(0), 64,
                 lambda x4, y0: x4[0:64, y0 + 1:y0 + 15, 0, 0:28]),
                (wk2(1), 64,
                 lambda x4, y0: x4[0:64, y0 + 1:y0 + 15, 1, 0:28]),
                (wk2(2), 64,
                 lambda x4, y0: x4[0:64, y0 + 1:y0 + 15, 0, 1:29]),
            ]

        # ---------------- phase A: conv1 + convd ----------------
        with tc.tile_pool(name="pc1", bufs=6, space="PSUM") as pc1, \
             tc.tile_pool(name="pcd", bufs=2, space="PSUM") as pcd:

            def emit_convd(n, x4):
                psd = {h: pcd.tile([128, NBLK], F32, tag="pcd",
                                   name=f"psd_{n}_{h}")
                       for h in range(2)}
                for h in range(2):
                    nc.tensor.matmul(psd[h], wdk,
                                     x4[0:64, 14 * h:14 * h + 14, 0, 0:28],
                                     start=True, stop=True)
                for h in range(2):
                    y0 = 14 * h
                    blk = 2 * n + h
                    dst = cd_t[n][:, y0 * 28:(y0 + 14) * 28]
                    if h == 0:
                        nc.vector.tensor_copy(dst, psd[h][:])
                    else:
                        nc.scalar.copy(dst, psd[h][:])
                    nc.vector.bn_stats(statsd[:, 6 * blk:6 * blk + 6], dst)

            for n0 in range(0, PER, 2):
                pair = (n0, n0 + 1)
                x4s, pss = {}, {}
                for n in pair:
                    xt = xpool.tile([128, XFREE], BF16, tag="xt")
                    nc.sync.dma_start(xt[:], xin[n * 128:(n + 1) * 128, :])
                    x4s[n] = xt.rearrange("p (r t x) -> p r t x",
                                          r=29, t=2, x=29)
                    c1_t.append(c1pool.tile([128, NPIX], BF16, tag="c1", name=f"c1_{n}"))
                    cd_t.append(cdpool.tile([128, NPIX], BF16, tag="cd", name=f"cd_{n}"))

                blocks = [(n, h) for n in pair for h in range(2)]
                for nh in blocks:
                    pss[nh] = pc1.tile([128, NBLK], F32, tag="pc1",
                                       name=f"ps1_{nh[0]}_{nh[1]}")
                # taps outer, blocks inner: consecutive matmuls share lhsT
                taps = c1_taps()
                for t, (w_ap, kp, rhs_fn) in enumerate(taps):
                    for (n, h) in blocks:
                        nc.tensor.matmul(pss[(n, h)], w_ap,
                                         rhs_fn(x4s[n], 14 * h),
                                         start=(t == 0),
                                         stop=(t == len(taps) - 1))
                for (n, h) in blocks:
                    y0 = 14 * h
                    blk = 2 * n + h
                    dst = c1_t[n][:, y0 * 28:(y0 + 14) * 28]
                    nc.scalar.copy(dst, pss[(n, h)][:])
                    nc.vector.bn_stats(stats1[:, 6 * blk:6 * blk + 6], dst)

                # convd inline for all but the last pair; the last pair's
                # convd is deferred to the phase A/B boundary so the PE has
                # work while DVE/ACT run the BN1 coefficient chain.
                if n0 + 2 < PER:
                    for n in pair:
                        emit_convd(n, x4s[n])
                else:
                    deferred = [(n, x4s[n]) for n in pair]

            for n, x4 in deferred:
                emit_convd(n, x4)

        # ---- BN1 coefficients (per-shard stats, no sync) ----
        mv1 = coef[:, 0:2]
        nc.vector.bn_aggr(mv1, stats1[:])
        nc.vector.tensor_scalar_add(coef[:, 2:3], mv1[:, 1:2], EPS)
        nc.scalar.sqrt(coef[:, 3:4], coef[:, 2:3])
        nc.vector.reciprocal(coef[:, 4:5], coef[:, 3:4])      # inv1
        s1 = coef[:, 5:6]
        t1 = coef[:, 6:7]
        nc.vector.tensor_mul(s1, gb_t[:, 0:1], coef[:, 4:5])
        nc.vector.tensor_mul(coef[:, 7:8], mv1[:, 0:1], s1)
        nc.vector.tensor_sub(t1, gb_t[:, 1:2], coef[:, 7:8])

        # ---- BNd stats -> AllReduce (overlaps phase B) ----
        mvd = coef[:, 8:10]
        nc.vector.bn_aggr(mvd, statsd[:])
        ar_d = constp.tile([128, 2], F32, tag="ard")
        nc.vector.tensor_copy(ar_d[:, 0:1], mvd[:, 0:1])
        nc.vector.tensor_mul(ar_d[:, 1:2], mvd[:, 0:1], mvd[:, 0:1])
        nc.vector.tensor_add(ar_d[:, 1:2], ar_d[:, 1:2], mvd[:, 1:2])
        bd_in = drp.tile([128, 2], F32, tag="bdi")
        bd_out = drp.tile([128, 2], F32, addr_space="Shared", tag="bdo")
        nc.sync.dma_start(bd_in[:], ar_d[:])
        nc.gpsimd.collective_compute(
            "AllReduce", _ADD, replica_groups=_GROUPS,
            ins=[bd_in.opt()], outs=[bd_out.opt()])
        ard_g = constp.tile([128, 2], F32, tag="ardg")
        nc.sync.dma_start(ard_g[:], bd_out[:])
        nc.vector.tensor_scalar_mul(ard_g[:], ard_g[:], 1.0 / N_CORES)
        nc.vector.tensor_mul(coef[:, 10:11], ard_g[:, 0:1], ard_g[:, 0:1])
        nc.vector.tensor_sub(coef[:, 11:12], ard_g[:, 1:2], coef[:, 10:11])
        nc.vector.tensor_scalar_add(coef[:, 12:13], coef[:, 11:12], EPS)
        nc.scalar.sqrt(coef[:, 13:14], coef[:, 12:13])
        nc.vector.reciprocal(coef[:, 14:15], coef[:, 13:14])  # invd
        sd = coef[:, 15:16]
        td = coef[:, 16:17]
        nc.vector.tensor_mul(sd, gb_t[:, 2:3], coef[:, 14:15])
        nc.vector.tensor_mul(coef[:, 17:18], ard_g[:, 0:1], sd)
        nc.vector.tensor_sub(td, gb_t[:, 3:4], coef[:, 17:18])

        # ---------------- phase B: bn1+relu, conv2 ----------------
        taps9 = [(1, 1)] + [(kh, kw) for kh in range(3)
                            for kw in range(3) if (kh, kw) != (1, 1)]
        with tc.tile_pool(name="pc2", bufs=6, space="PSUM") as pc2:
            for n0 in range(0, PER, 2):
                pair = (n0, n0 + 1)
                yvs = {}
                for n in pair:
                    yv = y1_t[n].rearrange("p (r x) -> p r x", x=30)
                    nc.scalar.activation(yv[:, 1:29, 1:29],
                                         c1_t[n].rearrange(
                                             "p (r x) -> p r x", x=28),
                                         _RELU, bias=t1, scale=s1)
                    yvs[n] = yv
                    c2_t.append(c2pool.tile([128, NPIX], BF16, tag="c2", name=f"c2_{n}"))
                blocks = [(n, h) for n in pair for h in range(2)]
                pss = {nh: pc2.tile([128, NBLK], F32, tag="pc2",
                                    name=f"ps2_{nh[0]}_{nh[1]}")
                       for nh in blocks}
                for t, (kh, kw) in enumerate(taps9):
                    for (n, h) in blocks:
                        y0 = 14 * h
                        rhs = yvs[n][:, y0 + kh:y0 + kh + 14, kw:kw + 28]
                        nc.tensor.matmul(pss[(n, h)], w2k(kh, kw), rhs,
                                         start=(t == 0),
                                         stop=(t == len(taps9) - 1))
                for (n, h) in blocks:
                    y0 = 14 * h
                    blk = 2 * n + h
                    dst = c2_t[n][:, y0 * 28:(y0 + 14) * 28]
                    nc.scalar.copy(dst, pss[(n, h)][:])
                    nc.vector.bn_stats(stats2[:, 6 * blk:6 * blk + 6], dst)

        # v = bnd(cd), in place.  Placed after the conv2 loop so the
        # statically-ordered DVE stream never stalls on the BNd AllReduce
        # mid-phase; by now the collective has long completed.
        for n in range(PER):
            nc.vector.tensor_scalar(cd_t[n][:], cd_t[n][:], sd, td,
                                    _MULT, _ADD)

        # ---- BN2 stats -> AllReduce ----
        mv2 = coef[:, 18:20]
        nc.vector.bn_aggr(mv2, stats2[:])
        ar_2 = constp.tile([128, 2], F32, tag="ar2")
        nc.vector.tensor_copy(ar_2[:, 0:1], mv2[:, 0:1])
        nc.vector.tensor_mul(ar_2[:, 1:2], mv2[:, 0:1], mv2[:, 0:1])
        nc.vector.tensor_add(ar_2[:, 1:2], ar_2[:, 1:2], mv2[:, 1:2])
        b2_in = drp.tile([128, 2], F32, tag="b2i")
        b2_out = drp.tile([128, 2], F32, addr_space="Shared", tag="b2o")
        nc.sync.dma_start(b2_in[:], ar_2[:])
        nc.gpsimd.collective_compute(
            "AllReduce", _ADD, replica_groups=_GROUPS,
            ins=[b2_in.opt()], outs=[b2_out.opt()])
        ar2_g = constp.tile([128, 2], F32, tag="ar2g")
        nc.sync.dma_start(ar2_g[:], b2_out[:])
        nc.vector.tensor_scalar_mul(ar2_g[:], ar2_g[:], 1.0 / N_CORES)
        nc.vector.tensor_mul(coef[:, 20:21], ar2_g[:, 0:1], ar2_g[:, 0:1])
        nc.vector.tensor_sub(coef[:, 21:22], ar2_g[:, 1:2], coef[:, 20:21])
        nc.vector.tensor_scalar_add(coef[:, 22:23], coef[:, 21:22], EPS)
        nc.scalar.sqrt(coef[:, 23:24], coef[:, 22:23])
        s2 = coef[:, 18:19]   # reuse mv2 columns (consumed above)
        t2 = coef[:, 19:20]
        nc.vector.reciprocal(coef[:, 21:22], coef[:, 23:24])  # inv2
        nc.vector.tensor_mul(s2, gb_t[:, 4:5], coef[:, 21:22])
        nc.vector.tensor_mul(coef[:, 20:21], ar2_g[:, 0:1], s2)
        nc.vector.tensor_sub(t2, gb_t[:, 5:6], coef[:, 20:21])

        # ---------------- phase C: combine + relu + store ----------------
        _MAX = mybir.AluOpType.max
        for n in range(PER):
            zf = zpool.tile([128, NPIX], BF16, tag="zf")
            nc.vector.scalar_tensor_tensor(zf[:], c2_t[n][:], s2, cd_t[n][:],
                                           _MULT, _ADD)
            og = opool.tile([128, NPIX], F32, tag="og")
            if n % 2 == 0:
                nc.scalar.activation(og[:], zf[:], _RELU, bias=t2)
            else:
                nc.vector.tensor_scalar(og[:], zf[:], t2, 0.0, _ADD, _MAX)
            nc.sync.dma_start(out[n * 128:(n + 1) * 128, :], og[:])


def build_nc():
    nc = bacc.Bacc("TRN2", target_bir_lowering=False, debug=False,
                   num_devices=N_CORES)
    xin = nc.dram_tensor("xin", [PER * 128, XFREE], BF16,
                         kind="ExternalInput").ap()
    wts = nc.dram_tensor("wts", [128, 2048], BF16, kind="ExternalInput").ap()
    gb = nc.dram_tensor("gb", [128, 8], F32, kind="ExternalInput").ap()
    out = nc.dram_tensor("out", [PER * 128, NPIX], F32,
                         kind="ExternalOutput").ap()
    with tile.TileContext(nc) as tc:
        _kernel_body(tc, nc, xin, wts, gb, out)
    nc.compile()
    return nc


def prep_inputs(x, w1, g1, b1, w2, g2, b2, wd, gd, bd):
    """Host-side shard + layout prep. Returns in_maps for the 8 cores."""
    x = np.asarray(x, dtype=np.float32)
    # even/odd row split on partitions, even/odd column split inside each
    # row: free = [row(29)][parity(2)][x(29)], data rows 0..27 / x 0..27
    xp = np.zeros((B, 128, 29, 2, 29), dtype=np.float32)
    xp[:, 0:64, 0:28, 0, 0:28] = x[:, :, 0::2, 0::2]
    xp[:, 0:64, 0:28, 1, 0:28] = x[:, :, 0::2, 1::2]
    xp[:, 64:128, 0:28, 0, 0:28] = x[:, :, 1::2, 0::2]
    xp[:, 64:128, 0:28, 1, 0:28] = x[:, :, 1::2, 1::2]
    xp = xp.reshape(B, 128, XFREE).astype(BF16NP)

    w1 = np.asarray(w1, dtype=np.float32)
    w2 = np.asarray(w2, dtype=np.float32)
    wd = np.asarray(wd, dtype=np.float32)
    w_all = np.zeros((128, 16, 128), dtype=np.float32)
    for t in range(3):
        w_all[0:64, t, :] = w1[:, :, 0, t].T
        w_all[64:128, t, :] = w1[:, :, 1, t].T
        w_all[0:64, 3 + t, :] = w1[:, :, 2, t].T
    w_all[0:64, 6, :] = wd[:, :, 0, 0].T
    for kh in range(3):
        for kw in range(3):
            w_all[:, 7 + 3 * kh + kw, :] = w2[:, :, kh, kw].T
    w_all = w_all.reshape(128, 2048).astype(BF16NP)

    gbm = np.zeros((128, 8), dtype=np.float32)
    for j, v in enumerate([g1, b1, gd, bd, g2, b2]):
        gbm[:, j] = np.asarray(v, dtype=np.float32)

    in_maps = []
    for c in range(N_CORES):
        shard = xp[c * PER:(c + 1) * PER].reshape(PER * 128, XFREE)
        in_maps.append({"xin": np.ascontiguousarray(shard),
                        "wts": w_all, "gb": gbm})
    return in_maps


_NC_CACHE = None


def kernel(**inputs):
    global _NC_CACHE
    if _NC_CACHE is None:
        _NC_CACHE = build_nc()
    nc = _NC_CACHE
    in_maps = prep_inputs(**inputs)
    res = bass_utils.run_bass_kernel_spmd(
        nc, in_maps, core_ids=list(range(N_CORES)))
    outs = [res.results[c]["out"].reshape(PER, COUT, OH, OW)
            for c in range(N_CORES)]
    return np.ascontiguousarray(np.concatenate(outs, axis=0),
                                dtype=np.float32)


# revision 12
# speedup vs baseline: 1.3097x; 1.1171x over previous
"""Trainium2 Bass kernel for a ResNet BasicBlock (stride-2, downsample) in
BatchNorm training mode.

  out = relu(bn2(conv2(relu(bn1(conv1(x))))) + bnd(convd(x)))
  conv1: 3x3 s2 SAME, conv2: 3x3 s1 SAME, convd: 1x1 s2 VALID
  x: (128, 64, 56, 56) f32 -> out: (128, 128, 28, 28) f32

Sharding: data-parallel over batch across 8 NeuronCores (16 images each),
weights replicated.  BN1 uses per-shard batch stats (sanctioned by the
sharding hint; the downstream BN2 re-normalization absorbs most of the
shard-stat error).  BNd and BN2 stats are made exact (full-batch) with one
tiny AllReduce each of the per-core (mean, E[x^2]) vectors.

Convs run as shift-and-accumulate matmuls in bf16 with f32 PSUM
accumulation.  x is pre-packed on the host into an even/odd row- and
column-split layout (zero padding baked in) so every tap's moving operand
is contiguous in its innermost dim (strided operands stream ~60% slower
and do not register as PE activity for the HAM clock un-throttle) and the
(kh=0, kh=1) tap pairs contract over K=128.  Matmuls are ordered
taps-outer over image pairs so consecutive matmuls share the stationary
operand (weight reloads amortize).
"""

import os
import sys

import numpy as np

try:
    import concourse.bass as bass
except ImportError:  # fall back to the staged repo location
    for _p in ("/opt/trn_rl_repo", "/root/.axon_site/_ro/trn_rl_repo"):
        if _p not in sys.path:
            sys.path.insert(0, _p)
    import concourse.bass as bass

import ml_dtypes
import concourse.bacc as bacc
import concourse.mybir as mybir
import concourse.tile as tile
from concourse import bass_utils

F32 = mybir.dt.float32
BF16 = mybir.dt.bfloat16
BF16NP = ml_dtypes.bfloat16

N_CORES = 8
B, CIN, H, W = 128, 64, 56, 56
COUT, OH, OW = 128, 28, 28
PER = B // N_CORES          # images per core
XFREE = 29 * 58             # row-split block: 29 rows x (2 parities x 29 x)
NPIX = OH * OW              # 784
NBLK = 392                  # one half-image block: 14 rows x 28 cols
NB = 2 * PER                # stat blocks per conv (two per image)
Y1F = 30 * 30               # padded y1 layout
EPS = 1e-5

_ADD = mybir.AluOpType.add
_MULT = mybir.AluOpType.mult
_RELU = mybir.ActivationFunctionType.Relu
_GROUPS = [list(range(N_CORES))]


def _kernel_body(tc, nc, xin, wts, gb, out):
    with tc.tile_pool(name="const", bufs=1) as constp, \
         tc.tile_pool(name="xs", bufs=4) as xpool, \
         tc.tile_pool(name="c1p", bufs=PER) as c1pool, \
         tc.tile_pool(name="cdp", bufs=PER) as cdpool, \
         tc.tile_pool(name="c2p", bufs=PER) as c2pool, \
         tc.tile_pool(name="y1p", bufs=PER) as y1pool, \
         tc.tile_pool(name="zfp", bufs=3) as zpool, \
         tc.tile_pool(name="ogp", bufs=3) as opool, \
         tc.tile_pool(name="dram", bufs=1, space="DRAM") as drp:

        w_t = constp.tile([128, 2048], BF16, tag="w")
        nc.scalar.dma_start(w_t[:], wts[:])
        gb_t = constp.tile([128, 8], F32, tag="gb")
        nc.sync.dma_start(gb_t[:], gb[:])

        stats1 = constp.tile([128, 6 * NB], F32, tag="st1")
        statsd = constp.tile([128, 6 * NB], F32, tag="std")
        stats2 = constp.tile([128, 6 * NB], F32, tag="st2")
        coef = constp.tile([128, 24], F32, tag="coef")

        def w01(t):
            return w_t[:, t * 128:(t + 1) * 128]

        def wk2(t):
            return w_t[0:64, (3 + t) * 128:(4 + t) * 128]

        wdk = w_t[0:64, 6 * 128:7 * 128]

        def w2k(kh, kw):
            t = 7 + 3 * kh + kw
            return w_t[:, t * 128:(t + 1) * 128]

        c1_t, cd_t, c2_t, y1_t = [], [], [], []

        # y1 tiles are persistent and zero-padded once; the BN1 activation
        # only ever writes the 28x28 interior, so the pad ring stays zero.
        for n in range(PER):
            y1n = y1pool.tile([128, Y1F], BF16, tag="y1")
            y1_t.append(y1n)
            nc.gpsimd.memset(y1n[:], 0.0)

        # conv1 taps: (weight AP, partition count, rhs slice builder).
        # x4 dims: [p, row(29), parity(2), x(29)] -- row 28 / x 28 are pads.
        def c1_taps():
            return [
                (w01(0), 128, lambda x4, y0: x4[:, y0:y0 + 14, 0, 0:28]),
                (w01(1), 128, lambda x4, y0: x4[:, y0:y0 + 14, 1, 0:28]),
                (w01(2), 128, lambda x4, y0: x4[:, y0:y0 + 14, 0, 1:29]),
                (wk2(0), 64,
                 lambda x4, y0: x4[0:64, y0 + 1:y0 + 15, 0, 0:28]),
                (wk2(1), 64,
                 lambda x4, y0: x4[0:64, y0 + 1:y0 + 15, 1, 0:28]),
                (wk2(2), 64,
                 lambda x4, y0: x4[0:64, y0 + 1:y0 + 15, 0, 1:29]),
            ]

        # ---------------- phase A: conv1 + convd ----------------
        with tc.tile_pool(name="pc1", bufs=6, space="PSUM") as pc1, \
             tc.tile_pool(name="pcd", bufs=2, space="PSUM") as pcd:

            def emit_convd(n, x4):
                psd = {h: pcd.tile([128, NBLK], F32, tag="pcd",
                                   name=f"psd_{n}_{h}")
                       for h in range(2)}
                for h in range(2):
                    nc.tensor.matmul(psd[h], wdk,
                                     x4[0:64, 14 * h:14 * h + 14, 0, 0:28],
                                     start=True, stop=True)
                for h in range(2):
                    y0 = 14 * h
                    blk = 2 * n + h
                    dst = cd_t[n][:, y0 * 28:(y0 + 14) * 28]
                    if h == 0:
                        nc.vector.tensor_copy(dst, psd[h][:])
                    else:
                        nc.scalar.copy(dst, psd[h][:])
                    nc.vector.bn_stats(statsd[:, 6 * blk:6 * blk + 6], dst)

            for n0 in range(0, PER, 2):
                pair = (n0, n0 + 1)
                x4s, pss = {}, {}
                for n in pair:
                    xt = xpool.tile([128, XFREE], BF16, tag="xt")
                    nc.sync.dma_start(xt[:], xin[n * 128:(n + 1) * 128, :])
                    x4s[n] = xt.rearrange("p (r t x) -> p r t x",
                                          r=29, t=2, x=29)
                    c1_t.append(c1pool.tile([128, NPIX], BF16, tag="c1", name=f"c1_{n}"))
                    cd_t.append(cdpool.tile([128, NPIX], BF16, tag="cd", name=f"cd_{n}"))

                blocks = [(n, h) for n in pair for h in range(2)]
                for nh in blocks:
                    pss[nh] = pc1.tile([128, NBLK], F32, tag="pc1",
                                       name=f"ps1_{nh[0]}_{nh[1]}")
                # taps outer, blocks inner: consecutive matmuls share lhsT
                taps = c1_taps()
                for t, (w_ap, kp, rhs_fn) in enumerate(taps):
                    for (n, h) in blocks:
                        nc.tensor.matmul(pss[(n, h)], w_ap,
                                         rhs_fn(x4s[n], 14 * h),
                                         start=(t == 0),
                                         stop=(t == len(taps) - 1))
                for (n, h) in blocks:
                    y0 = 14 * h
                    blk = 2 * n + h
                    dst = c1_t[n][:, y0 * 28:(y0 + 14) * 28]
                    nc.scalar.copy(dst, pss[(n, h)][:])
                    nc.vector.bn_stats(stats1[:, 6 * blk:6 * blk + 6], dst)

                # convd inline for all but the last pair; the last pair's
                # convd is deferred to the phase A/B boundary so the PE has
                # work while DVE/ACT run the BN1 coefficient chain.
                if n0 + 2 < PER:
                    for n in pair:
                        emit_convd(n, x4s[n])
                else:
                    deferred = [(n, x4s[n]) for n in pair]

            for n, x4 in deferred:
                emit_convd(n, x4)

        # ---- BN1 coefficients (per-shard stats, no sync) ----
        mv1 = coef[:, 0:2]
        nc.vector.bn_aggr(mv1, stats1[:])
        nc.vector.tensor_scalar_add(coef[:, 2:3], mv1[:, 1:2], EPS)
        nc.scalar.sqrt(coef[:, 3:4], coef[:, 2:3])
        nc.vector.reciprocal(coef[:, 4:5], coef[:, 3:4])      # inv1
        s1 = coef[:, 5:6]
        t1 = coef[:, 6:7]
        nc.vector.tensor_mul(s1, gb_t[:, 0:1], coef[:, 4:5])
        nc.vector.tensor_mul(coef[:, 7:8], mv1[:, 0:1], s1)
        nc.vector.tensor_sub(t1, gb_t[:, 1:2], coef[:, 7:8])

        # ---- BNd stats -> AllReduce (overlaps phase B) ----
        mvd = coef[:, 8:10]
        nc.vector.bn_aggr(mvd, statsd[:])
        ar_d = constp.tile([128, 2], F32, tag="ard")
        nc.vector.tensor_copy(ar_d[:, 0:1], mvd[:, 0:1])
        nc.vector.tensor_mul(ar_d[:, 1:2], mvd[:, 0:1], mvd[:, 0:1])
        nc.vector.tensor_add(ar_d[:, 1:2], ar_d[:, 1:2], mvd[:, 1:2])
        bd_in = drp.tile([128, 2], F32, tag="bdi")
        bd_out = drp.tile([128, 2], F32, addr_space="Shared", tag="bdo")
        nc.sync.dma_start(bd_in[:], ar_d[:])
        nc.gpsimd.collective_compute(
            "AllReduce", _ADD, replica_groups=_GROUPS,
            ins=[bd_in.opt()], outs=[bd_out.opt()])
        # NOTE: everything that CONSUMES the ARd result is emitted after the
        # conv2 loop.  The first collective can take ~60us; any op waiting on
        # it that lands early in a statically-ordered engine queue would
        # head-block that engine mid-phase-B and stall the PE via the psum
        # pools.

        # ---------------- phase B: bn1+relu, conv2 ----------------
        taps9 = [(1, 1)] + [(kh, kw) for kh in range(3)
                            for kw in range(3) if (kh, kw) != (1, 1)]
        with tc.tile_pool(name="pc2", bufs=6, space="PSUM") as pc2:
            for n0 in range(0, PER, 2):
                pair = (n0, n0 + 1)
                yvs = {}
                for n in pair:
                    yv = y1_t[n].rearrange("p (r x) -> p r x", x=30)
                    nc.scalar.activation(yv[:, 1:29, 1:29],
                                         c1_t[n].rearrange(
                                             "p (r x) -> p r x", x=28),
                                         _RELU, bias=t1, scale=s1)
                    yvs[n] = yv
                    c2_t.append(c2pool.tile([128, NPIX], BF16, tag="c2", name=f"c2_{n}"))
                blocks = [(n, h) for n in pair for h in range(2)]
                pss = {nh: pc2.tile([128, NBLK], F32, tag="pc2",
                                    name=f"ps2_{nh[0]}_{nh[1]}")
                       for nh in blocks}
                for t, (kh, kw) in enumerate(taps9):
                    for (n, h) in blocks:
                        y0 = 14 * h
                        rhs = yvs[n][:, y0 + kh:y0 + kh + 14, kw:kw + 28]
                        nc.tensor.matmul(pss[(n, h)], w2k(kh, kw), rhs,
                                         start=(t == 0),
                                         stop=(t == len(taps9) - 1))
                for (n, h) in blocks:
                    y0 = 14 * h
                    blk = 2 * n + h
                    dst = c2_t[n][:, y0 * 28:(y0 + 14) * 28]
                    nc.scalar.copy(dst, pss[(n, h)][:])
                    nc.vector.bn_stats(stats2[:, 6 * blk:6 * blk + 6], dst)

        # ---- BNd coefficients (consumes the ARd result) ----
        ard_g = constp.tile([128, 2], F32, tag="ardg")
        nc.sync.dma_start(ard_g[:], bd_out[:])
        nc.vector.tensor_scalar_mul(ard_g[:], ard_g[:], 1.0 / N_CORES)
        nc.vector.tensor_mul(coef[:, 10:11], ard_g[:, 0:1], ard_g[:, 0:1])
        nc.vector.tensor_sub(coef[:, 11:12], ard_g[:, 1:2], coef[:, 10:11])
        nc.vector.tensor_scalar_add(coef[:, 12:13], coef[:, 11:12], EPS)
        nc.scalar.sqrt(coef[:, 13:14], coef[:, 12:13])
        nc.vector.reciprocal(coef[:, 14:15], coef[:, 13:14])  # invd
        sd = coef[:, 15:16]
        td = coef[:, 16:17]
        nc.vector.tensor_mul(sd, gb_t[:, 2:3], coef[:, 14:15])
        nc.vector.tensor_mul(coef[:, 17:18], ard_g[:, 0:1], sd)
        nc.vector.tensor_sub(td, gb_t[:, 3:4], coef[:, 17:18])

        # v = bnd(cd), in place
        for n in range(PER):
            nc.vector.tensor_scalar(cd_t[n][:], cd_t[n][:], sd, td,
                                    _MULT, _ADD)

        # ---- BN2 stats -> AllReduce ----
        mv2 = coef[:, 18:20]
        nc.vector.bn_aggr(mv2, stats2[:])
        ar_2 = constp.tile([128, 2], F32, tag="ar2")
        nc.vector.tensor_copy(ar_2[:, 0:1], mv2[:, 0:1])
        nc.vector.tensor_mul(ar_2[:, 1:2], mv2[:, 0:1], mv2[:, 0:1])
        nc.vector.tensor_add(ar_2[:, 1:2], ar_2[:, 1:2], mv2[:, 1:2])
        b2_in = drp.tile([128, 2], F32, tag="b2i")
        b2_out = drp.tile([128, 2], F32, addr_space="Shared", tag="b2o")
        nc.sync.dma_start(b2_in[:], ar_2[:])
        nc.gpsimd.collective_compute(
            "AllReduce", _ADD, replica_groups=_GROUPS,
            ins=[b2_in.opt()], outs=[b2_out.opt()])
        ar2_g = constp.tile([128, 2], F32, tag="ar2g")
        nc.sync.dma_start(ar2_g[:], b2_out[:])
        nc.vector.tensor_scalar_mul(ar2_g[:], ar2_g[:], 1.0 / N_CORES)
        nc.vector.tensor_mul(coef[:, 20:21], ar2_g[:, 0:1], ar2_g[:, 0:1])
        nc.vector.tensor_sub(coef[:, 21:22], ar2_g[:, 1:2], coef[:, 20:21])
        nc.vector.tensor_scalar_add(coef[:, 22:23], coef[:, 21:22], EPS)
        nc.scalar.sqrt(coef[:, 23:24], coef[:, 22:23])
        s2 = coef[:, 18:19]   # reuse mv2 columns (consumed above)
        t2 = coef[:, 19:20]
        nc.vector.reciprocal(coef[:, 21:22], coef[:, 23:24])  # inv2
        nc.vector.tensor_mul(s2, gb_t[:, 4:5], coef[:, 21:22])
        nc.vector.tensor_mul(coef[:, 20:21], ar2_g[:, 0:1], s2)
        nc.vector.tensor_sub(t2, gb_t[:, 5:6], coef[:, 20:21])

        # ---------------- phase C: combine + relu + store ----------------
        _MAX = mybir.AluOpType.max
        for n in range(PER):
            zf = zpool.tile([128, NPIX], BF16, tag="zf")
            nc.vector.scalar_tensor_tensor(zf[:], c2_t[n][:], s2, cd_t[n][:],
                                           _MULT, _ADD)
            og = opool.tile([128, NPIX], F32, tag="og")
            if n % 2 == 0:
                nc.scalar.activation(og[:], zf[:], _RELU, bias=t2)
            else:
                nc.vector.tensor_scalar(og[:], zf[:], t2, 0.0, _ADD, _MAX)
            nc.sync.dma_start(out[n * 128:(n + 1) * 128, :], og[:])


def build_nc():
    nc = bacc.Bacc("TRN2", target_bir_lowering=False, debug=False,
                   num_devices=N_CORES)
    xin = nc.dram_tensor("xin", [PER * 128, XFREE], BF16,
                         kind="ExternalInput").ap()
    wts = nc.dram_tensor("wts", [128, 2048], BF16, kind="ExternalInput").ap()
    gb = nc.dram_tensor("gb", [128, 8], F32, kind="ExternalInput").ap()
    out = nc.dram_tensor("out", [PER * 128, NPIX], F32,
                         kind="ExternalOutput").ap()
    with tile.TileContext(nc) as tc:
        _kernel_body(tc, nc, xin, wts, gb, out)
    nc.compile()
    return nc


def prep_inputs(x, w1, g1, b1, w2, g2, b2, wd, gd, bd):
    """Host-side shard + layout prep. Returns in_maps for the 8 cores."""
    x = np.asarray(x, dtype=np.float32)
    # even/odd row split on partitions, even/odd column split inside each
    # row: free = [row(29)][parity(2)][x(29)], data rows 0..27 / x 0..27
    xp = np.zeros((B, 128, 29, 2, 29), dtype=np.float32)
    xp[:, 0:64, 0:28, 0, 0:28] = x[:, :, 0::2, 0::2]
    xp[:, 0:64, 0:28, 1, 0:28] = x[:, :, 0::2, 1::2]
    xp[:, 64:128, 0:28, 0, 0:28] = x[:, :, 1::2, 0::2]
    xp[:, 64:128, 0:28, 1, 0:28] = x[:, :, 1::2, 1::2]
    xp = xp.reshape(B, 128, XFREE).astype(BF16NP)

    w1 = np.asarray(w1, dtype=np.float32)
    w2 = np.asarray(w2, dtype=np.float32)
    wd = np.asarray(wd, dtype=np.float32)
    w_all = np.zeros((128, 16, 128), dtype=np.float32)
    for t in range(3):
        w_all[0:64, t, :] = w1[:, :, 0, t].T
        w_all[64:128, t, :] = w1[:, :, 1, t].T
        w_all[0:64, 3 + t, :] = w1[:, :, 2, t].T
    w_all[0:64, 6, :] = wd[:, :, 0, 0].T
    for kh in range(3):
        for kw in range(3):
            w_all[:, 7 + 3 * kh + kw, :] = w2[:, :, kh, kw].T
    w_all = w_all.reshape(128, 2048).astype(BF16NP)

    gbm = np.zeros((128, 8), dtype=np.float32)
    for j, v in enumerate([g1, b1, gd, bd, g2, b2]):
        gbm[:, j] = np.asarray(v, dtype=np.float32)

    in_maps = []
    for c in range(N_CORES):
        shard = xp[c * PER:(c + 1) * PER].reshape(PER * 128, XFREE)
        in_maps.append({"xin": np.ascontiguousarray(shard),
                        "wts": w_all, "gb": gbm})
    return in_maps


_NC_CACHE = None


def kernel(**inputs):
    global _NC_CACHE
    if _NC_CACHE is None:
        _NC_CACHE = build_nc()
    nc = _NC_CACHE
    in_maps = prep_inputs(**inputs)
    res = bass_utils.run_bass_kernel_spmd(
        nc, in_maps, core_ids=list(range(N_CORES)))
    outs = [res.results[c]["out"].reshape(PER, COUT, OH, OW)
            for c in range(N_CORES)]
    return np.ascontiguousarray(np.concatenate(outs, axis=0),
                                dtype=np.float32)
